# revision 9
# baseline (speedup 1.0000x reference)
"""LIF spike recurrence kernel for Trainium2 (8 NeuronCores, SPMD). v6.

Problem: x [32, 128, 32, 32, 8] f32, recurrence over last (time) dim:
    u_t = TAU * u_{t-1} * (1 - o_{t-1}) + x_t
    o_t = 1[u_t - VTH > 0]
Output: o [32, 128, 32, 32, 8] f32 (0.0 / 1.0 spikes).

Design (all facts hardware-probed):
  - Shard batch (32) across 8 cores -> 4/core; host pre-transposes each shard
    to plane-major [P=128, T=8, NPP=4096] so every SBUF access is contiguous
    (strided fp32 STT costs ~1.6x; contiguous runs at (FD+151)/0.96 ns exact,
    back-to-back with ~40ns gaps).
  - One mega-tile [P, T, NPP] per core; input DMA chunked (planes 0/1 in
    quarters, interleaved, so step-1 compute starts ~4 us earlier).
  - Exact fp32 recurrence on DVE, bit-identical to the reference:
       c   = (u_{t-1} <= VTH) * u_{t-1}     (STT is_le/mult; x{0,1} exact)
       u_t = c * TAU + x_t   in place       (TAU=2^-2 exact; single round)
  - Spike via ScalarE: o8_t = Sign(u_t - VTH) -> int8 {-1,0,1}, one ACTIVATE
    per plane ((FD+352)/1.2 ns, no bubble), fully hidden under DVE. Host maps
    >0 to 1.0f (exact). int8 output cuts out-DMA 4x vs f32.
  - Output DMA rides the Scalar HWDGE ring (input uses Sync's) so the queues
    never serialize.
  - Steps 1 and 7 are column-quartered to shorten pipeline head/tail; the
    engine-start barrier only orders ScalarE after the bias-const memset.
"""

import numpy as np

TAU = 0.25
VTH = 0.3
N_CORES = 8
P = 128
T = 8
B_LOC = 4  # batches per core
PIX_PER_CORE = B_LOC * 128 * 32 * 32  # 524288
NPP = PIX_PER_CORE // P  # 4096 pixels per partition

_CACHE = {}


def _build_nc():
    import concourse.tile as tile
    from concourse import bacc, mybir

    f32 = mybir.dt.float32
    i8 = mybir.dt.int8
    Alu = mybir.AluOpType
    AF = mybir.ActivationFunctionType

    nc = bacc.Bacc(
        "TRN2",
        target_bir_lowering=False,
        debug=False,
        enable_asserts=False,
        num_devices=N_CORES,
    )
    x_d = nc.dram_tensor("x", [P, T, NPP], f32, kind="ExternalInput").ap()
    o_d = nc.dram_tensor("o", [P, T, NPP], i8, kind="ExternalOutput").ap()

    # ACT activation bias needs a pre-registered const AP. Only ScalarE reads
    # it, so barrier just {GpSimd -> Scalar}; Sync starts its DMAs unblocked.
    cb = nc.alloc_sbuf_tensor("const-f32-negvth", [128, 1], f32)
    nc.gpsimd.memset(cb.ap(), -VTH)
    nc.const_aps.aps[(f32, -VTH)] = cb.ap()
    nc.multi_engine_barrier([mybir.EngineType.Pool, mybir.EngineType.Activation])

    with tile.TileContext(nc) as tc:
        with tc.tile_pool(name="pp", bufs=1) as pp:
            xt = pp.tile([P, T, NPP], f32, tag="xt")
            c = pp.tile([P, NPP], f32, tag="c")
            o8 = pp.tile([P, T, NPP], i8, tag="o8")

            Q = NPP // 4
            E = NPP // 8
            # Planes 0/1 arrive in interleaved quarters (first quarter as two
            # eighths so the very first compute op starts earliest); rest whole.
            nc.sync.dma_start(xt[:, 0, :E], x_d[:, 0, :E])
            nc.sync.dma_start(xt[:, 0, E:Q], x_d[:, 0, E:Q])
            nc.sync.dma_start(xt[:, 1, :E], x_d[:, 1, :E])
            nc.sync.dma_start(xt[:, 1, E:Q], x_d[:, 1, E:Q])
            for q in range(1, 4):
                nc.sync.dma_start(xt[:, 0, q * Q : (q + 1) * Q],
                                  x_d[:, 0, q * Q : (q + 1) * Q])
                nc.sync.dma_start(xt[:, 1, q * Q : (q + 1) * Q],
                                  x_d[:, 1, q * Q : (q + 1) * Q])
            for t in range(2, T):
                nc.sync.dma_start(xt[:, t, :], x_d[:, t, :])

            def cu(t, sl):
                up = xt[:, t - 1, sl]
                nc.vector.scalar_tensor_tensor(
                    c[:, sl], up, VTH, up, op0=Alu.is_le, op1=Alu.mult
                )
                nc.vector.scalar_tensor_tensor(
                    xt[:, t, sl], c[:, sl], TAU, xt[:, t, sl],
                    op0=Alu.mult, op1=Alu.add,
                )

            # Plane 0: u_0 = x_0, spike immediately.
            nc.scalar.activation(o8[:, 0, :], xt[:, 0, :], AF.Sign, bias=-VTH)
            nc.scalar.dma_start(o_d[:, 0, :], o8[:, 0, :])

            # Step 1: chases the chunked DMAs (eighths first, then quarters).
            cu(1, slice(0, E))
            cu(1, slice(E, Q))
            for q in range(1, 4):
                cu(1, slice(q * Q, (q + 1) * Q))
            nc.scalar.activation(o8[:, 1, :], xt[:, 1, :], AF.Sign, bias=-VTH)
            nc.scalar.dma_start(o_d[:, 1, :], o8[:, 1, :])

            # Steps 2..T-2: full-plane ops (minimal op count).
            for t in range(2, T - 1):
                cu(t, slice(0, NPP))
                nc.scalar.activation(o8[:, t, :], xt[:, t, :], AF.Sign, bias=-VTH)
                nc.scalar.dma_start(o_d[:, t, :], o8[:, t, :])

            # Step T-1: quartered with interleaved signs/outs (short tail).
            for q in range(4):
                sl = slice(q * Q, (q + 1) * Q)
                cu(T - 1, sl)
                nc.scalar.activation(
                    o8[:, T - 1, sl], xt[:, T - 1, sl], AF.Sign, bias=-VTH
                )
                nc.scalar.dma_start(o_d[:, T - 1, sl], o8[:, T - 1, sl])
    nc.compile()
    return nc


def _get_nc():
    if "nc" not in _CACHE:
        _CACHE["nc"] = _build_nc()
    return _CACHE["nc"]


def _shard(x: np.ndarray):
    xs = np.ascontiguousarray(x, dtype=np.float32)
    return [
        np.ascontiguousarray(
            xs[i * B_LOC : (i + 1) * B_LOC].reshape(P, NPP, T).transpose(0, 2, 1)
        )
        for i in range(N_CORES)
    ]


def _run(in_maps, **kwargs):
    from concourse.bass_utils import run_bass_kernel_spmd

    nc = _get_nc()
    return run_bass_kernel_spmd(nc, in_maps, core_ids=list(range(N_CORES)), **kwargs)


def kernel(x: np.ndarray) -> np.ndarray:
    in_maps = [{"x": s} for s in _shard(x)]
    res = _run(in_maps)
    outs = []
    for i in range(N_CORES):
        s8 = res.results[i]["o"]  # [P, T, NPP] int8 sign values
        o = (s8 > 0).transpose(0, 2, 1).astype(np.float32)  # [P, NPP, T]
        outs.append(o.reshape(B_LOC, 128, 32, 32, T))
    return np.concatenate(outs, axis=0)


# revision 11
# speedup vs baseline: 1.0004x; 1.0004x over previous
"""LIF spike recurrence kernel for Trainium2 (8 NeuronCores, SPMD). v6.

Problem: x [32, 128, 32, 32, 8] f32, recurrence over last (time) dim:
    u_t = TAU * u_{t-1} * (1 - o_{t-1}) + x_t
    o_t = 1[u_t - VTH > 0]
Output: o [32, 128, 32, 32, 8] f32 (0.0 / 1.0 spikes).

Design (all facts hardware-probed):
  - Shard batch (32) across 8 cores -> 4/core; host pre-transposes each shard
    to plane-major [P=128, T=8, NPP=4096] so every SBUF access is contiguous
    (strided fp32 STT costs ~1.6x; contiguous runs at (FD+151)/0.96 ns exact,
    back-to-back with ~40ns gaps).
  - One mega-tile [P, T, NPP] per core; input DMA chunked (planes 0/1 in
    quarters, interleaved, so step-1 compute starts ~4 us earlier).
  - Exact fp32 recurrence on DVE, bit-identical to the reference:
       c   = (u_{t-1} <= VTH) * u_{t-1}     (STT is_le/mult; x{0,1} exact)
       u_t = c * TAU + x_t   in place       (TAU=2^-2 exact; single round)
  - Spike via ScalarE: o8_t = Sign(u_t - VTH) -> int8 {-1,0,1}, one ACTIVATE
    per plane ((FD+352)/1.2 ns, no bubble), fully hidden under DVE. Host maps
    >0 to 1.0f (exact). int8 output cuts out-DMA 4x vs f32.
  - Output DMA rides the Scalar HWDGE ring (input uses Sync's) so the queues
    never serialize.
  - Steps 1 and 7 are column-quartered to shorten pipeline head/tail; the
    engine-start barrier only orders ScalarE after the bias-const memset.
"""

import numpy as np

TAU = 0.25
VTH = 0.3
N_CORES = 8
P = 128
T = 8
B_LOC = 4  # batches per core
PIX_PER_CORE = B_LOC * 128 * 32 * 32  # 524288
NPP = PIX_PER_CORE // P  # 4096 pixels per partition

_CACHE = {}


def _build_nc():
    import concourse.tile as tile
    from concourse import bacc, mybir

    f32 = mybir.dt.float32
    i8 = mybir.dt.int8
    Alu = mybir.AluOpType
    AF = mybir.ActivationFunctionType

    nc = bacc.Bacc(
        "TRN2",
        target_bir_lowering=False,
        debug=False,
        enable_asserts=False,
        num_devices=N_CORES,
    )
    x_d = nc.dram_tensor("x", [P, T, NPP], f32, kind="ExternalInput").ap()
    o_d = nc.dram_tensor("o", [P, T, NPP], i8, kind="ExternalOutput").ap()

    # ACT activation bias needs a pre-registered const AP. Only ScalarE reads
    # it, so barrier just {GpSimd -> Scalar}; Sync starts its DMAs unblocked.
    cb = nc.alloc_sbuf_tensor("const-f32-negvth", [128, 1], f32)
    nc.gpsimd.memset(cb.ap(), -VTH)
    nc.const_aps.aps[(f32, -VTH)] = cb.ap()
    nc.multi_engine_barrier([mybir.EngineType.Pool, mybir.EngineType.Activation])

    with tile.TileContext(nc) as tc:
        with tc.tile_pool(name="pp", bufs=1) as pp:
            xt = pp.tile([P, T, NPP], f32, tag="xt")
            c = pp.tile([P, NPP], f32, tag="c")
            o8 = pp.tile([P, T, NPP], i8, tag="o8")

            Q = NPP // 4
            # Planes 0/1 arrive in interleaved quarters; rest whole.
            for q in range(4):
                nc.sync.dma_start(xt[:, 0, q * Q : (q + 1) * Q],
                                  x_d[:, 0, q * Q : (q + 1) * Q])
                nc.sync.dma_start(xt[:, 1, q * Q : (q + 1) * Q],
                                  x_d[:, 1, q * Q : (q + 1) * Q])
            for t in range(2, T):
                nc.sync.dma_start(xt[:, t, :], x_d[:, t, :])

            def cu(t, sl):
                up = xt[:, t - 1, sl]
                nc.vector.scalar_tensor_tensor(
                    c[:, sl], up, VTH, up, op0=Alu.is_le, op1=Alu.mult
                )
                nc.vector.scalar_tensor_tensor(
                    xt[:, t, sl], c[:, sl], TAU, xt[:, t, sl],
                    op0=Alu.mult, op1=Alu.add,
                )

            # Plane 0: u_0 = x_0, spike immediately.
            nc.scalar.activation(o8[:, 0, :], xt[:, 0, :], AF.Sign, bias=-VTH)
            nc.scalar.dma_start(o_d[:, 0, :], o8[:, 0, :])

            # Step 1: quartered (chases the quarter DMAs).
            for q in range(4):
                cu(1, slice(q * Q, (q + 1) * Q))
            nc.scalar.activation(o8[:, 1, :], xt[:, 1, :], AF.Sign, bias=-VTH)
            nc.scalar.dma_start(o_d[:, 1, :], o8[:, 1, :])

            # Steps 2..T-2: full-plane ops (minimal op count).
            for t in range(2, T - 1):
                cu(t, slice(0, NPP))
                nc.scalar.activation(o8[:, t, :], xt[:, t, :], AF.Sign, bias=-VTH)
                nc.scalar.dma_start(o_d[:, t, :], o8[:, t, :])

            # Step T-1: quartered with interleaved signs/outs (short tail).
            for q in range(4):
                sl = slice(q * Q, (q + 1) * Q)
                cu(T - 1, sl)
                nc.scalar.activation(
                    o8[:, T - 1, sl], xt[:, T - 1, sl], AF.Sign, bias=-VTH
                )
                nc.scalar.dma_start(o_d[:, T - 1, sl], o8[:, T - 1, sl])
    nc.compile()
    return nc


def _get_nc():
    if "nc" not in _CACHE:
        _CACHE["nc"] = _build_nc()
    return _CACHE["nc"]


def _shard(x: np.ndarray):
    xs = np.ascontiguousarray(x, dtype=np.float32)
    return [
        np.ascontiguousarray(
            xs[i * B_LOC : (i + 1) * B_LOC].reshape(P, NPP, T).transpose(0, 2, 1)
        )
        for i in range(N_CORES)
    ]


def _run(in_maps, **kwargs):
    from concourse.bass_utils import run_bass_kernel_spmd

    nc = _get_nc()
    return run_bass_kernel_spmd(nc, in_maps, core_ids=list(range(N_CORES)), **kwargs)


def kernel(x: np.ndarray) -> np.ndarray:
    in_maps = [{"x": s} for s in _shard(x)]
    res = _run(in_maps)
    outs = []
    for i in range(N_CORES):
        s8 = res.results[i]["o"]  # [P, T, NPP] int8 sign values
        o = (s8 > 0).transpose(0, 2, 1).astype(np.float32)  # [P, NPP, T]
        outs.append(o.reshape(B_LOC, 128, 32, 32, T))
    return np.concatenate(outs, axis=0)
